# revision 5
# baseline (speedup 1.0000x reference)
"""Trainium2 Bass kernel for nn_MixtureOfExpertsHead.

Strategy:
- Data-parallel: shard B=16384 tokens across 8 cores (2048 each); replicate
  weights. No collectives; host gathers per-core outputs.
- Per core, two half-passes of 1024 tokens. x^T (host-transposed) is resident
  in SBUF for the half; all weights stream from HBM once per half.
- Gate (x@Wg1 -> relu -> @Wg2 -> top2 softmax weights) runs in plain fp32 on
  the PE (4 cyc/row) so the top-2 expert selection matches the fp32 reference
  exactly: a single flipped selection costs ~2% output error, so the gate
  cannot run in bf16/f32r.
- Experts (dense, all 8, faithful to the reference's redundant compute) run in
  float32r: full bf16-rate matmuls (1 cyc/row at N>=512) with ~1e-4 relative
  error, well under tolerance since selection is decided by the fp32 gate.
- Expert second layer (O=1) is an M=1 matmul accumulating over H2 chunks.
- Top-2 weights computed vectorized over [128 tokens, 4 subtiles, 8 experts]
  tiles: m1 = max, m2 = max of masked, w_e = exp(l_e-m1)*(l_e>=m2), w /= sum.
- Combine: eo [8, 512] PE-transposed to [512, 8], multiplied by gate weights,
  row-summed -> out.
"""

import sys

sys.path.insert(0, "/opt/trn_rl_repo")

import numpy as np

import concourse.bacc as bacc
import concourse.mybir as mybir
import concourse.tile as tile
from concourse.bass_utils import run_bass_kernel_spmd
from concourse.masks import make_identity

B, H, E, KTOP, OD = 16384, 4096, 8, 2, 1
H2 = H // 2
NCORES = 8
TOK = B // NCORES      # tokens per core
HALF = TOK // 2        # tokens per half-pass
TT = 512               # token tile (matmul free dim)
NT = HALF // TT        # token tiles per half (2)
KC = H // 128          # contraction chunks over H (32)
KCH = KC // 2          # per kh-half (16)
MC = H2 // 128         # output chunks over H2 (16)
NSUB = TT // 128       # 128-token subtiles per token tile (4)

f32 = mybir.dt.float32
f32r = mybir.dt.float32r
AF = mybir.ActivationFunctionType
AX = mybir.AxisListType
ALU = mybir.AluOpType


def _build(rep: int = 1):
    nc = bacc.Bacc()
    xT = nc.declare_dram_parameter("xT", [H, TOK], f32r, isOutput=False)
    We1 = nc.declare_dram_parameter("We1", [E, H, H2], f32r, isOutput=False)
    Wg1 = nc.declare_dram_parameter("Wg1", [H, H2], f32, isOutput=False)
    Wg2r = nc.declare_dram_parameter("Wg2r", [128, MC, E], f32, isOutput=False)
    We2p = nc.declare_dram_parameter("We2p", [128, MC, E, E], f32r, isOutput=False)
    be1r = nc.declare_dram_parameter("be1r", [128, E, MC], f32, isOutput=False)
    bg1r = nc.declare_dram_parameter("bg1r", [128, MC], f32, isOutput=False)
    bg2b = nc.declare_dram_parameter("bg2b", [128, E], f32, isOutput=False)
    be2r = nc.declare_dram_parameter("be2r", [E, 1], f32, isOutput=False)
    out = nc.declare_dram_parameter("out", [TOK, 1], f32, isOutput=True)

    xr = xT.rearrange("(k p) t -> p k t", p=128)
    Wg1r = Wg1.rearrange("(k p) m -> p k m", p=128)
    We1r = We1.rearrange("e (k p) m -> p e k m", p=128)
    outr = out.rearrange("(c p) o -> p (c o)", p=128)

    with tile.TileContext(nc) as tc:
        with (
            tc.tile_pool(name="consts", bufs=1) as consts,
            tc.tile_pool(name="xp", bufs=1) as xp,
            tc.tile_pool(name="wgp", bufs=2) as wgp,
            tc.tile_pool(name="wep", bufs=3) as wep,
            tc.tile_pool(name="hp", bufs=3) as hp,
            tc.tile_pool(name="laccp", bufs=2) as laccp,
            tc.tile_pool(name="wtp", bufs=3) as wtp,
            tc.tile_pool(name="eosbp", bufs=3) as eosbp,
            tc.tile_pool(name="tops", bufs=2) as tops,
            tc.tile_pool(name="outp", bufs=1) as outp,
            tc.tile_pool(name="bigp", bufs=4, space="PSUM") as bigp,
            tc.tile_pool(name="smallp", bufs=2, space="PSUM") as smallp,
            tc.tile_pool(name="eop", bufs=2, space="PSUM") as eop,
        ):
            # ---- constants (loaded once) ----
            wg2_sb = consts.tile([128, MC, E], f32)
            nc.sync.dma_start(wg2_sb[:], Wg2r[:])
            we2_sb = consts.tile([128, MC, E, E], f32r)
            nc.sync.dma_start(we2_sb[:], We2p[:])
            be1_sb = consts.tile([128, E, MC], f32)
            nc.sync.dma_start(be1_sb[:], be1r[:])
            bg1_sb = consts.tile([128, MC], f32)
            nc.sync.dma_start(bg1_sb[:], bg1r[:])
            bg2_sb = consts.tile([128, E], f32)
            nc.sync.dma_start(bg2_sb[:], bg2b[:])
            be2_sb = consts.tile([E, 1], f32)
            nc.sync.dma_start(be2_sb[:], be2r[:])
            ident = consts.tile([128, 128], f32)
            make_identity(nc, ident[:])

            out_sb = outp.tile([128, TOK // 128], f32)

            for _r in range(rep):
                for h in range(2):
                    # ---- load resident x^T half: [128, KC, HALF] ----
                    x_sb = xp.tile([128, KC, HALF], f32r, tag="x")
                    for k in range(KC):
                        nc.sync.dma_start(
                            x_sb[:, k], xr[:, k, h * HALF:(h + 1) * HALF]
                        )

                    w_tiles = []  # per t: [128, NSUB, E] gate combine weights
                    # ======== GATE (plain fp32) ========
                    for t in range(NT):
                        lacc = laccp.tile([128, NSUB, E], f32, tag="lacc")
                        prev = None
                        for m in range(MC):
                            ps = bigp.tile([128, TT], f32, tag="big")
                            for kh in range(2):
                                wg = wgp.tile([128, KCH, 128], f32, tag="wg")
                                for q in range(4):
                                    nc.sync.dma_start(
                                        wg[:, q * 4:(q + 1) * 4],
                                        Wg1r[:, kh * KCH + q * 4:kh * KCH + (q + 1) * 4,
                                             m * 128:(m + 1) * 128],
                                    )
                                for k in range(KCH):
                                    nc.tensor.matmul(
                                        ps,
                                        lhsT=wg[:, k],
                                        rhs=x_sb[:, kh * KCH + k,
                                                 t * TT:(t + 1) * TT].bitcast(f32),
                                        start=(kh == 0 and k == 0),
                                        stop=(kh == 1 and k == KCH - 1),
                                    )
                            gh = hp.tile([128, TT], f32, tag="gh")
                            nc.scalar.activation(
                                gh[:], ps, AF.Relu, bias=bg1_sb[:, m:m + 1]
                            )
                            if prev is not None:
                                _logits_mms(nc, tc, smallp, prev[0], prev[1],
                                            wg2_sb, lacc, bg2_sb)
                            prev = (gh, m)
                        _logits_mms(nc, tc, smallp, prev[0], prev[1],
                                    wg2_sb, lacc, bg2_sb)

                        # ---- top-2 softmax weights, vectorized [128,NSUB,E] ----
                        shp = [128, NSUB, E]
                        m1 = tops.tile([128, NSUB, 1], f32, tag="m1")
                        nc.vector.reduce_max(m1[:], lacc[:], axis=AX.X)
                        d = tops.tile(shp, f32, tag="d")
                        nc.vector.tensor_tensor(
                            d[:], lacc[:], m1[:].to_broadcast(shp), ALU.subtract
                        )
                        eq = tops.tile(shp, f32, tag="eq")
                        nc.vector.tensor_scalar(
                            eq[:], d[:], 0.0, None, ALU.is_ge
                        )
                        masked = tops.tile(shp, f32, tag="masked")
                        nc.vector.scalar_tensor_tensor(
                            masked[:], eq[:], -1e30, d[:], ALU.mult, ALU.add
                        )
                        dm2 = tops.tile([128, NSUB, 1], f32, tag="dm2")
                        nc.vector.reduce_max(dm2[:], masked[:], axis=AX.X)
                        ex = tops.tile(shp, f32, tag="ex")
                        nc.scalar.activation(ex[:], d[:], AF.Exp)
                        mask2 = tops.tile(shp, f32, tag="mask2")
                        nc.vector.tensor_tensor(
                            mask2[:], d[:], dm2[:].to_broadcast(shp), ALU.is_ge
                        )
                        u = tops.tile(shp, f32, tag="u")
                        nc.vector.tensor_tensor(u[:], ex[:], mask2[:], ALU.mult)
                        s = tops.tile([128, NSUB, 1], f32, tag="s")
                        nc.vector.reduce_sum(s[:], u[:], axis=AX.X)
                        rinv = tops.tile([128, NSUB, 1], f32, tag="rinv")
                        nc.vector.reciprocal(rinv[:], s[:])
                        wt = wtp.tile(shp, f32, tag="wt")
                        nc.vector.tensor_tensor(
                            wt[:], u[:], rinv[:].to_broadcast(shp), ALU.mult
                        )
                        w_tiles.append(wt)

                    # ======== EXPERTS (f32r), t-tiles interleaved ========
                    eo_sb = [eosbp.tile([E, TT], f32, tag="eosb", name=f"eosb{_t}") for _t in range(NT)]
                    eo_ps = [eop.tile([E, TT], f32, tag="eo", name=f"eo{_t}") for _t in range(NT)]
                    for e in range(E):
                        prev = None
                        for m in range(MC):
                            ps = [bigp.tile([128, TT], f32, tag="big", name=f"hps{_t}") for _t in range(NT)]
                            for kh in range(2):
                                we = wep.tile([128, KCH, 128], f32r, tag="we")
                                for q in range(4):
                                    nc.sync.dma_start(
                                        we[:, q * 4:(q + 1) * 4],
                                        We1r[:, e, kh * KCH + q * 4:
                                             kh * KCH + (q + 1) * 4,
                                             m * 128:(m + 1) * 128],
                                    )
                                for t in range(NT):
                                    for k in range(KCH):
                                        nc.tensor.matmul(
                                            ps[t],
                                            lhsT=we[:, k],
                                            rhs=x_sb[:, kh * KCH + k,
                                                     t * TT:(t + 1) * TT],
                                            start=(kh == 0 and k == 0),
                                            stop=(kh == 1 and k == KCH - 1),
                                        )
                            hs = []
                            for t in range(NT):
                                ht = hp.tile([128, TT], f32r, tag="hs")
                                nc.scalar.activation(
                                    ht[:], ps[t], AF.Relu,
                                    bias=be1_sb[:, e, m:m + 1],
                                )
                                hs.append(ht)
                            if prev is not None:
                                _eo_mms(nc, eo_ps, we2_sb, e, prev[1], prev[0])
                            prev = (hs, m)
                        _eo_mms(nc, eo_ps, we2_sb, e, prev[1], prev[0])
                    for t in range(NT):
                        nc.scalar.activation(
                            eo_sb[t][:], eo_ps[t], AF.Identity,
                            bias=be2_sb[0:E, 0:1],
                        )

                    # ======== COMBINE ========
                    for t in range(NT):
                        for sub in range(NSUB):
                            tp = smallp.tile([128, E], f32, tag="small")
                            nc.tensor.transpose(
                                tp,
                                eo_sb[t][:, sub * 128:(sub + 1) * 128],
                                ident[0:E, 0:E],
                            )
                            prod = tops.tile([128, E], f32, tag="prod")
                            nc.vector.tensor_tensor(
                                prod[:], tp, w_tiles[t][:, sub, :], ALU.mult
                            )
                            gcol = h * (NT * NSUB) + t * NSUB + sub
                            nc.vector.reduce_sum(
                                out_sb[:, gcol:gcol + 1], prod[:], axis=AX.X
                            )

                nc.sync.dma_start(outr[:], out_sb[:])

    nc.compile()
    return nc


def _logits_mms(nc, tc, smallp, gh, m, wg2_sb, lacc, bg2_sb):
    """logits partial matmuls for gate-hidden chunk m, accumulated into lacc."""
    for sub in range(NSUB):
        lp = smallp.tile([128, E], f32, tag="small")
        nc.tensor.matmul(
            lp,
            lhsT=gh[:, sub * 128:(sub + 1) * 128],
            rhs=wg2_sb[:, m, :],
            start=True,
            stop=True,
        )
        if m == 0:
            nc.vector.tensor_tensor(lacc[:, sub, :], lp, bg2_sb[:], ALU.add)
        else:
            nc.vector.tensor_tensor(
                lacc[:, sub, :], lacc[:, sub, :], lp, ALU.add
            )


def _eo_mms(nc, eo_ps, we2_sb, e, m, hs):
    """expert-output (O=1, zero-padded to [*,E]) matmul for chunk m of expert e."""
    for t in range(len(eo_ps)):
        nc.tensor.matmul(
            eo_ps[t],
            lhsT=we2_sb[:, m, e, :],
            rhs=hs[t][:],
            start=(e == 0 and m == 0),
            stop=(e == E - 1 and m == MC - 1),
        )


_NC_CACHE = {}


def _get_nc(rep: int = 1):
    if rep not in _NC_CACHE:
        _NC_CACHE[rep] = _build(rep)
    return _NC_CACHE[rep]


def _prep_in_maps(inputs):
    x = np.ascontiguousarray(np.asarray(inputs["x"], dtype=np.float32))
    We1 = np.ascontiguousarray(np.asarray(inputs["We1"], dtype=np.float32))
    be1 = np.asarray(inputs["be1"], dtype=np.float32)
    We2 = np.asarray(inputs["We2"], dtype=np.float32)
    be2 = np.ascontiguousarray(np.asarray(inputs["be2"], dtype=np.float32))
    Wg1 = np.ascontiguousarray(np.asarray(inputs["Wg1"], dtype=np.float32))
    bg1 = np.asarray(inputs["bg1"], dtype=np.float32)
    Wg2 = np.asarray(inputs["Wg2"], dtype=np.float32)
    bg2 = np.asarray(inputs["bg2"], dtype=np.float32)

    Wg2r = np.ascontiguousarray(Wg2.reshape(MC, 128, E).transpose(1, 0, 2))
    We2p = np.zeros((128, MC, E, E), dtype=np.float32)
    for e in range(E):
        We2p[:, :, e, e] = We2[e, :, 0].reshape(MC, 128).T
    be1r = np.ascontiguousarray(be1.reshape(E, MC, 128).transpose(2, 0, 1))
    bg1r = np.ascontiguousarray(bg1.reshape(MC, 128).T)
    bg2b = np.ascontiguousarray(np.tile(bg2[None, :], (128, 1)))

    xT = np.ascontiguousarray(x.T)  # [H, B]
    shared = {
        "We1": We1, "Wg1": Wg1, "Wg2r": Wg2r, "We2p": We2p,
        "be1r": be1r, "bg1r": bg1r, "bg2b": bg2b, "be2r": be2,
    }
    in_maps = []
    for c in range(NCORES):
        m = dict(shared)
        m["xT"] = np.ascontiguousarray(xT[:, c * TOK:(c + 1) * TOK])
        in_maps.append(m)
    return in_maps


def kernel(**inputs) -> np.ndarray:
    in_maps = _prep_in_maps(inputs)
    nc = _get_nc(rep=1)
    res = run_bass_kernel_spmd(nc, in_maps, list(range(NCORES)))
    out = np.concatenate(
        [res.results[c]["out"] for c in range(NCORES)], axis=0
    ).astype(np.float32)
    return out


# revision 7
# speedup vs baseline: 968.1259x; 968.1259x over previous
"""Trainium2 Bass kernel for nn_MixtureOfExpertsHead.

Strategy:
- Data-parallel: shard B=16384 tokens across 8 cores (2048 each); replicate
  weights. No collectives; host gathers per-core outputs.
- Per core, four passes of 512 tokens. x^T (host-transposed, fp32) is resident
  in SBUF for the pass; a bf16 copy (DVE cast) feeds the expert matmuls; all
  weights stream from HBM each pass.
- Gate (x@Wg1 -> relu -> @Wg2 -> top2 softmax weights) runs in plain fp32 on
  the PE so the top-2 expert selection matches the fp32 reference to ~1e-6
  logit accuracy: a single flipped selection costs ~1.5% output error, so the
  gate cannot run in bf16/f32r (measured: f32r's ~3e-5 logit rounding flips
  several tokens on this dataset).
- Experts (dense, all 8, faithful to the reference's redundant compute) run in
  bf16 (full-rate matmuls, ~2e-3 relative error, well under tolerance since
  selection is decided by the fp32 gate).
- Expert second layer (O=1) uses zero-padded [*, E] We2 columns so every
  expert accumulates its output row into one [E, 512] PSUM bank (engine APs
  cannot target partition offsets that are not 32-aligned).
- Top-2 weights computed vectorized over [128 tokens, 4 subtiles, 8 experts]:
  m1 = max, m2 = max of masked, w_e = exp(l_e-m1)*(l_e>=m2), w /= sum(w).
- Combine: eo [8, 512] PE-transposed to [512, 8], multiplied by gate weights,
  row-summed -> out.
"""

import sys

sys.path.insert(0, "/opt/trn_rl_repo")

import ml_dtypes
import numpy as np

import concourse.bacc as bacc
import concourse.mybir as mybir
import concourse.tile as tile
from concourse.bass_utils import run_bass_kernel_spmd
from concourse.masks import make_identity

B, H, E, KTOP, OD = 16384, 4096, 8, 2, 1
H2 = H // 2
NCORES = 8
TOK = B // NCORES      # tokens per core (2048)
TT = 512               # tokens per pass (matmul free dim)
NP = TOK // TT         # passes per core (4)
KC = H // 128          # contraction chunks over H (32)
KCH = KC // 2          # per kh-half (16)
MC = H2 // 128         # output chunks over H2 (16)
NSUB = TT // 128       # 128-token subtiles per pass (4)

f32 = mybir.dt.float32
bf16 = mybir.dt.bfloat16
AF = mybir.ActivationFunctionType
AX = mybir.AxisListType
ALU = mybir.AluOpType


def _build(rep: int = 1):
    nc = bacc.Bacc()
    xT = nc.declare_dram_parameter("xT", [H, TOK], f32, isOutput=False)
    We1 = nc.declare_dram_parameter("We1", [E, H, H2], bf16, isOutput=False)
    Wg1 = nc.declare_dram_parameter("Wg1", [H, H2], f32, isOutput=False)
    Wg2r = nc.declare_dram_parameter("Wg2r", [128, MC, E], f32, isOutput=False)
    We2p = nc.declare_dram_parameter("We2p", [128, MC, E, E], bf16, isOutput=False)
    be1r = nc.declare_dram_parameter("be1r", [128, E, MC], f32, isOutput=False)
    bg1r = nc.declare_dram_parameter("bg1r", [128, MC], f32, isOutput=False)
    bg2b = nc.declare_dram_parameter("bg2b", [128, E], f32, isOutput=False)
    be2r = nc.declare_dram_parameter("be2r", [E, 1], f32, isOutput=False)
    out = nc.declare_dram_parameter("out", [TOK, 1], f32, isOutput=True)

    xr = xT.rearrange("(k p) t -> p k t", p=128)
    Wg1r = Wg1.rearrange("(k p) m -> p k m", p=128)
    We1r = We1.rearrange("e (k p) m -> p e k m", p=128)
    outr = out.rearrange("(c p) o -> p (c o)", p=128)

    with tile.TileContext(nc) as tc:
        with (
            tc.tile_pool(name="consts", bufs=1) as consts,
            tc.tile_pool(name="xp", bufs=1) as xp,
            tc.tile_pool(name="xbp", bufs=1) as xbp,
            tc.tile_pool(name="wgp", bufs=2) as wgp,
            tc.tile_pool(name="wep", bufs=4) as wep,
            tc.tile_pool(name="hp", bufs=3) as hp,
            tc.tile_pool(name="laccp", bufs=2) as laccp,
            tc.tile_pool(name="wtp", bufs=2) as wtp,
            tc.tile_pool(name="eosbp", bufs=2) as eosbp,
            tc.tile_pool(name="tops", bufs=2) as tops,
            tc.tile_pool(name="outp", bufs=1) as outp,
            tc.tile_pool(name="bigp", bufs=4, space="PSUM") as bigp,
            tc.tile_pool(name="smallp", bufs=2, space="PSUM") as smallp,
            tc.tile_pool(name="eop", bufs=2, space="PSUM") as eop,
        ):
            # ---- constants (loaded once) ----
            wg2_sb = consts.tile([128, MC, E], f32)
            nc.sync.dma_start(wg2_sb[:], Wg2r[:])
            we2_sb = consts.tile([128, MC, E, E], bf16)
            nc.sync.dma_start(we2_sb[:], We2p[:])
            be1_sb = consts.tile([128, E, MC], f32)
            nc.sync.dma_start(be1_sb[:], be1r[:])
            bg1_sb = consts.tile([128, MC], f32)
            nc.sync.dma_start(bg1_sb[:], bg1r[:])
            bg2_sb = consts.tile([128, E], f32)
            nc.sync.dma_start(bg2_sb[:], bg2b[:])
            be2_sb = consts.tile([E, 1], f32)
            nc.sync.dma_start(be2_sb[:], be2r[:])
            ident = consts.tile([128, 128], f32)
            make_identity(nc, ident[:])

            out_sb = outp.tile([128, TOK // 128], f32)

            for _r in range(rep):
                for p in range(NP):
                    # ---- load resident x^T pass tile + bf16 cast ----
                    x_sb = xp.tile([128, KC, TT], f32, tag="x")
                    for k in range(KC):
                        nc.sync.dma_start(
                            x_sb[:, k], xr[:, k, p * TT:(p + 1) * TT]
                        )
                    xb = xbp.tile([128, KC, TT], bf16, tag="xb")
                    nc.vector.tensor_copy(xb[:], x_sb[:])

                    # ======== GATE (plain fp32) ========
                    lacc = laccp.tile([128, NSUB, E], f32, tag="lacc")
                    prev = None
                    for m in range(MC):
                        ps = bigp.tile([128, TT], f32, tag="big")
                        for kh in range(2):
                            wg = wgp.tile([128, KCH, 128], f32, tag="wg")
                            for q in range(4):
                                nc.sync.dma_start(
                                    wg[:, q * 4:(q + 1) * 4],
                                    Wg1r[:, kh * KCH + q * 4:kh * KCH + (q + 1) * 4,
                                         m * 128:(m + 1) * 128],
                                )
                            for k in range(KCH):
                                nc.tensor.matmul(
                                    ps,
                                    lhsT=wg[:, k],
                                    rhs=x_sb[:, kh * KCH + k],
                                    start=(kh == 0 and k == 0),
                                    stop=(kh == 1 and k == KCH - 1),
                                )
                        gh = hp.tile([128, TT], f32, tag="gh")
                        nc.scalar.activation(
                            gh[:], ps, AF.Relu, bias=bg1_sb[:, m:m + 1]
                        )
                        if prev is not None:
                            _logits_mms(nc, smallp, prev[0], prev[1],
                                        wg2_sb, lacc, bg2_sb)
                        prev = (gh, m)
                    _logits_mms(nc, smallp, prev[0], prev[1],
                                wg2_sb, lacc, bg2_sb)

                    # ---- top-2 softmax weights, vectorized [128,NSUB,E] ----
                    shp = [128, NSUB, E]
                    m1 = tops.tile([128, NSUB, 1], f32, tag="m1")
                    nc.vector.reduce_max(m1[:], lacc[:], axis=AX.X)
                    d = tops.tile(shp, f32, tag="d")
                    nc.vector.tensor_tensor(
                        d[:], lacc[:], m1[:].to_broadcast(shp), ALU.subtract
                    )
                    eq = tops.tile(shp, f32, tag="eq")
                    nc.vector.tensor_scalar(eq[:], d[:], 0.0, None, ALU.is_ge)
                    masked = tops.tile(shp, f32, tag="masked")
                    nc.vector.scalar_tensor_tensor(
                        masked[:], eq[:], -1e30, d[:], ALU.mult, ALU.add
                    )
                    dm2 = tops.tile([128, NSUB, 1], f32, tag="dm2")
                    nc.vector.reduce_max(dm2[:], masked[:], axis=AX.X)
                    ex = tops.tile(shp, f32, tag="ex")
                    nc.scalar.activation(ex[:], d[:], AF.Exp)
                    mask2 = tops.tile(shp, f32, tag="mask2")
                    nc.vector.tensor_tensor(
                        mask2[:], d[:], dm2[:].to_broadcast(shp), ALU.is_ge
                    )
                    u = tops.tile(shp, f32, tag="u")
                    nc.vector.tensor_tensor(u[:], ex[:], mask2[:], ALU.mult)
                    s = tops.tile([128, NSUB, 1], f32, tag="s")
                    nc.vector.reduce_sum(s[:], u[:], axis=AX.X)
                    rinv = tops.tile([128, NSUB, 1], f32, tag="rinv")
                    nc.vector.reciprocal(rinv[:], s[:])
                    wt = wtp.tile(shp, f32, tag="wt")
                    nc.vector.tensor_tensor(
                        wt[:], u[:], rinv[:].to_broadcast(shp), ALU.mult
                    )

                    # ======== EXPERTS (bf16), accumulated into [E, TT] ========
                    eo_sb = eosbp.tile([E, TT], f32, tag="eosb")
                    eo_ps = eop.tile([E, TT], f32, tag="eo")
                    for e in range(E):
                        prev = None
                        for m in range(MC):
                            ps = bigp.tile([128, TT], f32, tag="big")
                            for kh in range(2):
                                we = wep.tile([128, KCH, 128], bf16, tag="we")
                                for q in range(4):
                                    nc.sync.dma_start(
                                        we[:, q * 4:(q + 1) * 4],
                                        We1r[:, e, kh * KCH + q * 4:
                                             kh * KCH + (q + 1) * 4,
                                             m * 128:(m + 1) * 128],
                                    )
                                for k in range(KCH):
                                    nc.tensor.matmul(
                                        ps,
                                        lhsT=we[:, k],
                                        rhs=xb[:, kh * KCH + k],
                                        start=(kh == 0 and k == 0),
                                        stop=(kh == 1 and k == KCH - 1),
                                    )
                            ht = hp.tile([128, TT], bf16, tag="hs")
                            nc.scalar.activation(
                                ht[:], ps, AF.Relu, bias=be1_sb[:, e, m:m + 1]
                            )
                            if prev is not None:
                                _eo_mm(nc, eo_ps, we2_sb, e, prev[1], prev[0])
                            prev = (ht, m)
                        _eo_mm(nc, eo_ps, we2_sb, e, prev[1], prev[0])
                    nc.scalar.activation(
                        eo_sb[:], eo_ps, AF.Identity, bias=be2_sb[0:E, 0:1]
                    )

                    # ======== COMBINE ========
                    for sub in range(NSUB):
                        tp = smallp.tile([128, E], f32, tag="small")
                        nc.tensor.transpose(
                            tp,
                            eo_sb[:, sub * 128:(sub + 1) * 128],
                            ident[0:E, 0:E],
                        )
                        prod = tops.tile([128, E], f32, tag="prod")
                        nc.vector.tensor_tensor(
                            prod[:], tp, wt[:, sub, :], ALU.mult
                        )
                        gcol = p * NSUB + sub
                        nc.vector.reduce_sum(
                            out_sb[:, gcol:gcol + 1], prod[:], axis=AX.X
                        )

                nc.sync.dma_start(outr[:], out_sb[:])

    nc.compile()
    return nc


def _logits_mms(nc, smallp, gh, m, wg2_sb, lacc, bg2_sb):
    """logits partial matmuls for gate-hidden chunk m, accumulated into lacc."""
    for sub in range(NSUB):
        lp = smallp.tile([128, E], f32, tag="small")
        nc.tensor.matmul(
            lp,
            lhsT=gh[:, sub * 128:(sub + 1) * 128],
            rhs=wg2_sb[:, m, :],
            start=True,
            stop=True,
        )
        if m == 0:
            nc.vector.tensor_tensor(lacc[:, sub, :], lp, bg2_sb[:], ALU.add)
        else:
            nc.vector.tensor_tensor(
                lacc[:, sub, :], lacc[:, sub, :], lp, ALU.add
            )


def _eo_mm(nc, eo_ps, we2_sb, e, m, ht):
    """expert-output (O=1, zero-padded to [*,E]) matmul for chunk m of expert e."""
    nc.tensor.matmul(
        eo_ps,
        lhsT=we2_sb[:, m, e, :],
        rhs=ht[:],
        start=(e == 0 and m == 0),
        stop=(e == E - 1 and m == MC - 1),
    )


_NC_CACHE = {}


def _get_nc(rep: int = 1):
    if rep not in _NC_CACHE:
        _NC_CACHE[rep] = _build(rep)
    return _NC_CACHE[rep]


def _prep_in_maps(inputs):
    x = np.ascontiguousarray(np.asarray(inputs["x"], dtype=np.float32))
    We1 = np.asarray(inputs["We1"], dtype=np.float32)
    be1 = np.asarray(inputs["be1"], dtype=np.float32)
    We2 = np.asarray(inputs["We2"], dtype=np.float32)
    be2 = np.ascontiguousarray(np.asarray(inputs["be2"], dtype=np.float32))
    Wg1 = np.ascontiguousarray(np.asarray(inputs["Wg1"], dtype=np.float32))
    bg1 = np.asarray(inputs["bg1"], dtype=np.float32)
    Wg2 = np.asarray(inputs["Wg2"], dtype=np.float32)
    bg2 = np.asarray(inputs["bg2"], dtype=np.float32)

    We1b = np.ascontiguousarray(We1.astype(ml_dtypes.bfloat16))
    Wg2r = np.ascontiguousarray(Wg2.reshape(MC, 128, E).transpose(1, 0, 2))
    We2p = np.zeros((128, MC, E, E), dtype=np.float32)
    for e in range(E):
        We2p[:, :, e, e] = We2[e, :, 0].reshape(MC, 128).T
    We2p = We2p.astype(ml_dtypes.bfloat16)
    be1r = np.ascontiguousarray(be1.reshape(E, MC, 128).transpose(2, 0, 1))
    bg1r = np.ascontiguousarray(bg1.reshape(MC, 128).T)
    bg2b = np.ascontiguousarray(np.tile(bg2[None, :], (128, 1)))

    xT = np.ascontiguousarray(x.T)  # [H, B]
    shared = {
        "We1": We1b, "Wg1": Wg1, "Wg2r": Wg2r, "We2p": We2p,
        "be1r": be1r, "bg1r": bg1r, "bg2b": bg2b, "be2r": be2,
    }
    in_maps = []
    for c in range(NCORES):
        m = dict(shared)
        m["xT"] = np.ascontiguousarray(xT[:, c * TOK:(c + 1) * TOK])
        in_maps.append(m)
    return in_maps


def kernel(**inputs) -> np.ndarray:
    in_maps = _prep_in_maps(inputs)
    nc = _get_nc(rep=1)
    res = run_bass_kernel_spmd(nc, in_maps, list(range(NCORES)))
    out = np.concatenate(
        [res.results[c]["out"] for c in range(NCORES)], axis=0
    ).astype(np.float32)
    return out
